# revision 28
# baseline (speedup 1.0000x reference)
"""Trainium2 Bass kernel for the attention-decoder greedy decode loop.

Sharding (8 cores, zero cross-core communication):
  - batch data-parallel: each core owns 8 of 64 batch rows end-to-end
    (attention, LSTM, full-vocab fc, argmax, greedy feedback).
  - fc_W (65MB) is streamed from HBM each step in 500-column chunks, with the
    first 5000 vocab columns resident in SBUF.

All sigmoids and the softmax exp are computed via tanh (single ACT table set).
LSTM state is kept doubled (H2=2h, Q=2c) so the 0.5 factors fold into
host-pre-scaled weights.
"""
import sys, os
sys.path.insert(0, "/opt/trn_rl_repo")
import numpy as np

import concourse.bass as bass
import concourse.bacc as bacc
import concourse.mybir as mybir
from concourse import tile
from concourse.bass_utils import run_bass_kernel_spmd

f32 = mybir.dt.float32
u32 = mybir.dt.uint32
AF = mybir.ActivationFunctionType
OP = mybir.AluOpType

NCORES = 8
V, ENC, DEC, EMB, ATT = 32000, 256, 256, 128, 256
T, N = 128, 64
NL = N // NCORES          # 8 local batch rows
VL = V // NCORES          # 4000 local vocab rows

_CACHE = {}


def build_kernel(L: int, reps: int = 1):
    nc = bacc.Bacc("TRN2", target_bir_lowering=False, debug=False, num_devices=NCORES)

    # ---------------- I/O ----------------
    enc_l = nc.dram_tensor("enc_l", [128, NL, ENC], f32, kind="ExternalInput")
    encT_l = nc.dram_tensor("encT_l", [128, 2, NL, 128], f32, kind="ExternalInput")
    WeT = nc.dram_tensor("WeT", [128, 2, ATT], f32, kind="ExternalInput")
    WdT = nc.dram_tensor("WdT", [128, 2, ATT], f32, kind="ExternalInput")
    WihT = nc.dram_tensor("WihT", [128, 3, 4 * DEC], f32, kind="ExternalInput")
    WhhT = nc.dram_tensor("WhhT", [128, 2, 4 * DEC], f32, kind="ExternalInput")
    fcWT = nc.dram_tensor("fcWT", [128, 4, V], f32, kind="ExternalInput")
    b_row = nc.dram_tensor("b_row", [1, 4 * DEC], f32, kind="ExternalInput")
    fcb_row = nc.dram_tensor("fcb_row", [1, V], f32, kind="ExternalInput")
    v_stat = nc.dram_tensor("v_stat", [128, 2], f32, kind="ExternalInput")
    ident = nc.dram_tensor("ident", [128, 128], f32, kind="ExternalInput")
    ones_row = nc.dram_tensor("ones_row", [1, N], f32, kind="ExternalInput")
    iota64 = nc.dram_tensor("iota64", [NL, V // 500], f32, kind="ExternalInput")
    emb_tab = nc.dram_tensor("emb_tab", [V, EMB], f32, kind="ExternalInput")
    y0_l = nc.dram_tensor("y0_l", [NL, 1], u32, kind="ExternalInput")

    logits_out = nc.dram_tensor("logits_out", [NL, L, V], f32, kind="ExternalOutput")
    atts_out = nc.dram_tensor("atts_out", [NL, L, T], f32, kind="ExternalOutput")
    with tile.TileContext(nc) as tc:
        with tc.tile_pool(name="const", bufs=1) as cp, \
             tc.tile_pool(name="work", bufs=2) as wp, \
             tc.tile_pool(name="big", bufs=1) as bp, \
             tc.tile_pool(name="ps_tp", bufs=2, space="PSUM") as pt, \
             tc.tile_pool(name="ps_sc", bufs=2, space="PSUM") as psc, \
             tc.tile_pool(name="ps_mm", bufs=2, space="PSUM") as pmm, \
             tc.tile_pool(name="dram", bufs=2, space="DRAM") as dp:

            # ---------------- load constants ----------------
            def cload(name, dram, shape):
                t_ = cp.tile(shape, f32, tag=name)
                nc.sync.dma_start(t_[:], dram[:])
                return t_

            enc_sb = cload("enc_sb", enc_l, [128, NL, ENC])
            WeT_sb = cload("WeT_sb", WeT, [128, 2, ATT])
            WdT_sb = cload("WdT_sb", WdT, [128, 2, ATT])
            WihT_sb = cload("WihT_sb", WihT, [128, 3, 4 * DEC])
            WhhT_sb = cload("WhhT_sb", WhhT, [128, 2, 4 * DEC])
            VRES = 4500   # vocab columns kept resident in SBUF
            fcres_sb = cp.tile([128, 4, VRES], f32, tag="fcres_sb")
            nc.sync.dma_start(fcres_sb[:], fcWT[:, :, 0:VRES])
            b_sb = cload("b_sb", b_row, [1, 4 * DEC])
            vstat_sb = cload("vstat_sb", v_stat, [128, 2])
            id_sb = cload("id_sb", ident, [128, 128])
            ones_sb = cload("ones_sb", ones_row, [1, N])
            iota64_sb = cload("iota64_sb", iota64, [NL, V // 500])
            y0_sb = cp.tile([NL, 1], u32, tag="y0_sb")
            nc.sync.dma_start(y0_sb[:], y0_l[:])

            # ---------------- prologue: e_projT[a, n, t] ----------------
            epT_sb = bp.tile([128, 2, NL, 128], f32, tag="epT")
            encT_view = encT_l[:].rearrange("p k n t -> p k (n t)")
            for ah in range(2):
                ep_ps = pmm.tile([128, NL * 128], f32, tag="mm2b")
                for half in range(2):
                    for ek in range(2):
                        ch = wp.tile([128, 512], f32, tag="encT_ch")
                        nc.sync.dma_start(ch[:], encT_view[:, ek, half * 512:(half + 1) * 512])
                        nc.tensor.matmul(
                            ep_ps[:, half * 512:(half + 1) * 512],
                            lhsT=WeT_sb[:, ek, ah * 128:(ah + 1) * 128],
                            rhs=ch[:],
                            start=(ek == 0), stop=(ek == 1))
                nc.vector.tensor_copy(
                    epT_sb[:, ah, :, :].rearrange("p n t -> p (n t)"), ep_ps[:])

            # zero initial state
            hT_prev = wp.tile([128, 2, NL], f32, tag="hT")
            Q_prev = wp.tile([NL, DEC], f32, tag="Q")
            nc.vector.memset(hT_prev[:], 0.0)
            nc.vector.memset(Q_prev[:], 0.0)
            y_prev = y0_sb

            for t in [tt for _ in range(reps) for tt in range(L)]:
                # ---------- emb gather (needs y_prev) ----------
                emb_sb = wp.tile([NL, EMB], f32, tag="emb")
                nc.gpsimd.indirect_dma_start(
                    out=emb_sb[:], out_offset=None, in_=emb_tab[:],
                    in_offset=bass.IndirectOffsetOnAxis(ap=y_prev[:, :1], axis=0))
                embT_ps = pt.tile([128, NL], f32, tag="tp")
                nc.tensor.transpose(out=embT_ps[:], in_=emb_sb[:], identity=id_sb[0:NL, 0:NL])
                xgT_sb = wp.tile([128, 3, NL], f32, tag="xgT")
                nc.vector.tensor_copy(xgT_sb[:, 0, :], embT_ps[:])

                # ---------- d_projT[a, n] = W_d @ h ----------
                dp_ps = pt.tile([128, 2, NL], f32, tag="tp")
                for ah in range(2):
                    for dk in range(2):
                        nc.tensor.matmul(
                            dp_ps[:, ah, :],
                            lhsT=WdT_sb[:, dk, ah * 128:(ah + 1) * 128],
                            rhs=hT_prev[:, dk, :],
                            start=(dk == 0), stop=(dk == 1))
                dp_sb = wp.tile([128, 2, NL], f32, tag="dp")
                nc.vector.tensor_copy(dp_sb[:], dp_ps[:])

                # ---------- u = tanh(e_projT + d_projT bcast) ----------
                u_sb = bp.tile([128, 2, NL, 128], f32, tag="u")
                for ah in range(2):
                    upre = wp.tile([128, NL, 128], f32, tag="upre")
                    nc.vector.tensor_tensor(
                        out=upre[:], in0=epT_sb[:, ah, :, :],
                        in1=dp_sb[:, ah, :].unsqueeze(2).to_broadcast([128, NL, 128]),
                        op=OP.add)
                    nc.scalar.activation(
                        out=u_sb[:, ah, :, :], in_=upre[:], func=AF.Tanh)

                # ---------- scores: s[n, t'] = sum_a v_w[a] u[a, n, t'] ----------
                # psum rows live on partition 0 only ([1, (n t)] flat), then
                # scatter-DMA to [n, t] layout for the softmax.
                sflat_sb = wp.tile([1, NL * 128], f32, tag="sflat")
                for half in range(2):
                    sp = psc.tile([1, 512], f32, tag="sc")
                    for ah in range(2):
                        nc.tensor.matmul(
                            sp[:],
                            lhsT=vstat_sb[:, ah:ah + 1],
                            rhs=u_sb[:, ah, :, :].rearrange("p n t -> p (n t)")[:, half * 512:(half + 1) * 512],
                            start=(ah == 0), stop=(ah == 1))
                    nc.scalar.copy(sflat_sb[:, half * 512:(half + 1) * 512], sp[:])
                scT_sb = wp.tile([NL, 128], f32, tag="scT")
                nc.sync.dma_start(scT_sb[:], sflat_sb[:])

                # ---------- softmax over t' (exp via tanh) ----------
                mx_ = wp.tile([NL, 1], f32, tag="mx")
                nc.vector.reduce_max(mx_[:], scT_sb[:], axis=mybir.AxisListType.X)
                nh_ = wp.tile([NL, 1], f32, tag="nh")
                nc.vector.tensor_scalar_mul(nh_[:], mx_[:], -0.5)
                tnh = wp.tile([NL, 128], f32, tag="tnh")
                nc.scalar.activation(out=tnh[:], in_=scT_sb[:], func=AF.Tanh,
                                     bias=nh_[:, :1], scale=0.5)
                num = wp.tile([NL, 128], f32, tag="num")
                nc.scalar.activation(out=num[:], in_=tnh[:], func=AF.Identity, bias=1.0)
                den = wp.tile([NL, 128], f32, tag="den")
                nc.vector.tensor_scalar(out=den[:], in0=tnh[:], scalar1=-1.0,
                                        scalar2=1.0, op0=OP.mult, op1=OP.add)
                rden = wp.tile([NL, 128], f32, tag="rden")
                nc.vector.reciprocal(rden[:], den[:])
                e_ = wp.tile([NL, 128], f32, tag="e_")
                nc.vector.tensor_tensor(out=e_[:], in0=num[:], in1=rden[:], op=OP.mult)
                S_ = wp.tile([NL, 1], f32, tag="S_")
                nc.vector.reduce_sum(S_[:], e_[:], axis=mybir.AxisListType.X)
                rS_ = wp.tile([NL, 1], f32, tag="rS_")
                nc.vector.reciprocal(rS_[:], S_[:])
                alT = wp.tile([NL, 128], f32, tag="alT")
                nc.scalar.activation(out=alT[:], in_=e_[:], func=AF.Copy, scale=rS_[:, :1])
                # atts output
                nc.sync.dma_start(atts_out[:, t:t + 1, :], alT[:].unsqueeze(1))

                # ---------- alpha[t', n] via transpose ----------
                al_ps = pt.tile([128, NL], f32, tag="tp")
                nc.tensor.transpose(out=al_ps[:], in_=alT[:], identity=id_sb[0:NL, 0:NL])
                al_sb = wp.tile([128, NL], f32, tag="al")
                nc.vector.tensor_copy(al_sb[:], al_ps[:])

                # ---------- ctxT[e, n] = sum_t enc[t, n, e] alpha[t, n] ----------
                # one matmul per (n, e-half) with enc slice stationary; output
                # lands directly in the transposed layout gates/fc need.
                for eh in range(2):
                    ctT_ps = pt.tile([128, NL], f32, tag="tp")
                    for n in range(NL):
                        nc.tensor.matmul(
                            ctT_ps[:, n:n + 1],
                            lhsT=enc_sb[:, n, eh * 128:(eh + 1) * 128],
                            rhs=al_sb[:, n:n + 1],
                            start=True, stop=True)
                    nc.vector.tensor_copy(xgT_sb[:, 1 + eh, :], ctT_ps[:])

                # ---------- ctx back to [n, e] layout (for the x AllGather) ----------
                ctx_sb = wp.tile([NL, ENC], f32, tag="ctx")
                for eh in range(2):
                    cx_ps = pt.tile([NL, 128], f32, tag="tp")
                    nc.tensor.transpose(out=cx_ps[:], in_=xgT_sb[:, 1 + eh, :],
                                        identity=id_sb[:, 0:128])
                    nc.scalar.copy(ctx_sb[:, eh * 128:(eh + 1) * 128], cx_ps[:])

                # ---------- gates = x@W_ih.T + h@W_hh.T + b  (order i,f,o,g) ----------
                g_ps = pmm.tile([NL, 4 * DEC], f32, tag="mm2b")
                for ch in range(2):
                    sl = slice(ch * 512, (ch + 1) * 512)
                    for k in range(3):
                        nc.tensor.matmul(g_ps[:, sl], lhsT=xgT_sb[:, k, :],
                                         rhs=WihT_sb[:, k, sl], start=(k == 0), stop=False)
                    for dk in range(2):
                        nc.tensor.matmul(g_ps[:, sl], lhsT=hT_prev[:, dk, :],
                                         rhs=WhhT_sb[:, dk, sl], start=False, stop=False)
                    nc.tensor.matmul(g_ps[:, sl], lhsT=ones_sb[0:1, 0:NL],
                                     rhs=b_sb[0:1, sl], start=False, stop=True)

                # ---------- LSTM pointwise (state H2=2h, Q=2c) ----------
                th_ifo = wp.tile([NL, 768], f32, tag="th_ifo")
                nc.scalar.activation(out=th_ifo[:], in_=g_ps[:, 0:768], func=AF.Tanh, scale=0.5)
                th_g = wp.tile([NL, 256], f32, tag="th_g")
                nc.scalar.activation(out=th_g[:], in_=g_ps[:, 768:1024], func=AF.Tanh)
                A2 = wp.tile([NL, 256], f32, tag="A2")
                nc.vector.scalar_tensor_tensor(out=A2[:], in0=th_ifo[:, 256:512], scalar=1.0,
                                               in1=Q_prev[:], op0=OP.add, op1=OP.mult)
                B2 = wp.tile([NL, 256], f32, tag="B2")
                nc.vector.scalar_tensor_tensor(out=B2[:], in0=th_ifo[:, 0:256], scalar=1.0,
                                               in1=th_g[:], op0=OP.add, op1=OP.mult)
                Q_new = wp.tile([NL, DEC], f32, tag="Q")
                nc.vector.scalar_tensor_tensor(out=Q_new[:], in0=A2[:], scalar=0.5,
                                               in1=B2[:], op0=OP.mult, op1=OP.add)
                tc2 = wp.tile([NL, 256], f32, tag="tc2")
                nc.scalar.activation(out=tc2[:], in_=Q_new[:], func=AF.Tanh, scale=0.5)
                h2_sb = wp.tile([NL, DEC], f32, tag="h2")
                nc.vector.scalar_tensor_tensor(out=h2_sb[:], in0=th_ifo[:, 512:768], scalar=1.0,
                                               in1=tc2[:], op0=OP.add, op1=OP.mult)

                # ---------- hT for next step ----------
                hT_new = wp.tile([128, 2, NL], f32, tag="hT")
                for dk in range(2):
                    hT_ps = pt.tile([128, NL], f32, tag="tp")
                    nc.tensor.transpose(out=hT_ps[:], in_=h2_sb[:, dk * 128:(dk + 1) * 128],
                                        identity=id_sb[0:NL, 0:NL])
                    nc.vector.tensor_copy(hT_new[:, dk, :], hT_ps[:])

                # ---------- fc input: xfT = [hT(2) ; ctxT(2)] (all local) ----------
                xfT_sb = wp.tile([128, 4, NL], f32, tag="xfT")
                nc.vector.tensor_copy(xfT_sb[:, 0, :], hT_new[:, 0, :])
                nc.vector.tensor_copy(xfT_sb[:, 1, :], hT_new[:, 1, :])
                nc.scalar.copy(xfT_sb[:, 2, :], xgT_sb[:, 1, :])
                nc.scalar.copy(xfT_sb[:, 3, :], xgT_sb[:, 2, :])

                # ---------- fc: stream fc_W, logits in 500-col chunks ----------
                NCH = V // 500
                NRES = VRES // 500
                vals_all = wp.tile([NL, NCH], f32, tag="vals_all")
                with_all = wp.tile([NL, NCH], f32, tag="with_all")
                for ch in range(NCH):
                    vs = ch * 500
                    bt = wp.tile([1, 500], f32, tag="bstream")
                    nc.sync.dma_start(bt[:], fcb_row[:, vs:vs + 500])
                    bsrc = bt[:]
                    if ch < NRES:
                        wsrc = fcres_sb[:, :, vs:vs + 500]
                    else:
                        wt = wp.tile([128, 4, 500], f32, tag="wstream")
                        nc.sync.dma_start(wt[:], fcWT[:, :, vs:vs + 500])
                        wsrc = wt[:]
                    f_ps = pmm.tile([NL, 500], f32, tag="mm2b")
                    for k in range(4):
                        nc.tensor.matmul(f_ps[:], lhsT=xfT_sb[:, k, :],
                                         rhs=wsrc[:, k, :] if ch < NRES else wsrc[:, k, :],
                                         start=(k == 0), stop=False)
                    nc.tensor.matmul(f_ps[:], lhsT=ones_sb[0:1, 0:NL],
                                     rhs=bsrc, start=False, stop=True)
                    lchunk = wp.tile([NL, 500], f32, tag="lchunk")
                    if ch % 2 == 0:
                        nc.scalar.copy(lchunk[:], f_ps[:])
                    else:
                        nc.vector.tensor_copy(lchunk[:], f_ps[:])
                    nc.sync.dma_start(
                        logits_out[:, t:t + 1, vs:vs + 500], lchunk[:].unsqueeze(1))
                    # per-chunk top-1 (value + within-chunk index)
                    cm8 = wp.tile([NL, 8], f32, tag="cm8")
                    nc.vector.max(out=cm8[:], in_=lchunk[:])
                    ci8 = wp.tile([NL, 8], u32, tag="ci8")
                    nc.vector.max_index(out=ci8[:], in_max=cm8[:], in_values=lchunk[:])
                    nc.vector.tensor_copy(vals_all[:, ch:ch + 1], cm8[:, 0:1])
                    nc.vector.tensor_copy(with_all[:, ch:ch + 1], ci8[:, 0:1])

                # ---------- global argmax over chunks ----------
                gm8 = wp.tile([NL, 8], f32, tag="gm8")
                nc.vector.max(out=gm8[:], in_=vals_all[:])
                gj8 = wp.tile([NL, 8], u32, tag="gj8")
                nc.vector.max_index(out=gj8[:], in_max=gm8[:], in_values=vals_all[:])
                jcf = wp.tile([NL, 1], f32, tag="jcf")
                nc.vector.tensor_copy(jcf[:], gj8[:, 0:1])
                mask = wp.tile([NL, NCH], f32, tag="mask")
                nc.vector.tensor_scalar(out=mask[:], in0=iota64_sb[:], scalar1=jcf[:, :1],
                                        scalar2=None, op0=OP.is_equal)
                ymul = wp.tile([NL, NCH], f32, tag="ymul")
                withel = wp.tile([NL, 1], f32, tag="withel")
                nc.vector.scalar_tensor_tensor(
                    out=ymul[:], in0=mask[:], scalar=1.0, in1=with_all[:],
                    op0=OP.mult, op1=OP.mult, accum_out=withel[:])
                yf = wp.tile([NL, 1], f32, tag="yf")
                nc.vector.tensor_scalar(out=yf[:], in0=jcf[:], scalar1=500.0,
                                        scalar2=None, op0=OP.mult)
                nc.vector.tensor_tensor(out=yf[:], in0=yf[:], in1=withel[:], op=OP.add)
                y_new = wp.tile([NL, 1], u32, tag="y")
                nc.vector.tensor_copy(y_new[:], yf[:])

                hT_prev = hT_new
                Q_prev = Q_new
                y_prev = y_new

    nc.compile()
    return nc


def _prep_inputs(enc_out, emb, W_ih, b_ih, W_hh, b_hh, W_e, W_d, v_w, fc_W, fc_b):
    """Host-side slicing / transposition / pre-scaling. Returns list of in_maps."""
    perm = np.concatenate([np.arange(0, 256), np.arange(256, 512),
                           np.arange(768, 1024), np.arange(512, 768)])  # i,f,g,o -> i,f,o,g
    Wih_p = W_ih[perm]                   # [1024, 384]
    Whh_p = W_hh[perm] * 0.5             # h-state doubled
    b_p = (b_ih + b_hh)[perm]

    WihT = np.ascontiguousarray(Wih_p.T.reshape(3, 128, 1024).transpose(1, 0, 2))
    WhhT = np.ascontiguousarray(Whh_p.T.reshape(2, 128, 1024).transpose(1, 0, 2))
    WeT = np.ascontiguousarray(W_e.T.reshape(2, 128, ATT).transpose(1, 0, 2))
    WdT = np.ascontiguousarray((W_d * 0.5).T.reshape(2, 128, ATT).transpose(1, 0, 2))
    v_stat = np.ascontiguousarray(v_w.reshape(2, 128).T)
    fw = fc_W.T.copy()                   # [512, 32000]
    fw[0:256] *= 0.5                     # h-state doubled
    fcWT = np.ascontiguousarray(fw.reshape(4, 128, V).transpose(1, 0, 2))
    fcb_row = np.ascontiguousarray(fc_b.reshape(1, -1))
    iota64 = np.broadcast_to(np.arange(V // 500, dtype=np.float32), (NL, V // 500)).copy()
    b_row = np.ascontiguousarray(b_p.reshape(1, -1))
    identm = np.eye(128, dtype=np.float32)
    ones_row = np.ones((1, N), np.float32)
    emb_c = np.ascontiguousarray(emb)

    in_maps = []
    for c in range(NCORES):
        nsl = slice(c * NL, (c + 1) * NL)
        vsl = slice(c * VL, (c + 1) * VL)
        encl = np.ascontiguousarray(enc_out[:, nsl, :])              # [128, 8, 256]
        encT = np.ascontiguousarray(
            encl.transpose(2, 1, 0).reshape(2, 128, NL, 128).transpose(1, 0, 2, 3))
        in_maps.append({
            "enc_l": encl,
            "encT_l": encT,
            "WeT": WeT, "WdT": WdT, "WihT": WihT, "WhhT": WhhT,
            "fcWT": fcWT,
            "b_row": b_row,
            "fcb_row": fcb_row,
            "v_stat": v_stat,
            "ident": identm,
            "ones_row": ones_row,
            "iota64": iota64,
            "emb_tab": emb_c,
            "y0_l": np.ones((NL, 1), np.uint32),
        })
    return in_maps


def kernel(enc_out, emb, W_ih, b_ih, W_hh, b_hh, W_e, W_d, v_w, fc_W, fc_b, max_len,
           _want_profile=False, _profile_kwargs=None):
    L = int(max_len)
    args = [np.asarray(a, np.float32) for a in
            (enc_out, emb, W_ih, b_ih, W_hh, b_hh, W_e, W_d, v_w, fc_W, fc_b)]
    in_maps = _prep_inputs(*args)

    if L not in _CACHE:
        _CACHE[L] = build_kernel(L)
    nc = _CACHE[L]

    kwargs = dict(_profile_kwargs or {})
    res = run_bass_kernel_spmd(nc, in_maps, core_ids=list(range(NCORES)),
                               trace=_want_profile, **kwargs)
    logits = np.concatenate([res.results[c]["logits_out"] for c in range(NCORES)], axis=0)
    atts = np.concatenate([res.results[c]["atts_out"] for c in range(NCORES)], axis=0)
    if _want_profile:
        kernel._last_result = res
    return logits, atts
